# revision 1
# baseline (speedup 1.0000x reference)
"""MoE gate (router) kernel for Trainium2.

Computes, for hidden_states [T, H] and gate weight [E, H]:
    logits = hidden_states @ weight.T          # [T, E]
    probs  = softmax(logits, axis=-1)
    topk_weight, topk_idx = top_k(probs, 8)    # normalized over the top-8
    row_idx = arange(T*8).reshape(8, T).T

Strategy (8 NeuronCores, data parallel over tokens):
  - Host pre-transposes: each core receives hsT [H, T/8] and wT [H, E] so the
    contraction dim H lands on SBUF partitions with fully-contiguous DMA --
    no on-device transposes at all.
  - fp32 accuracy from fp16 hi/lo splits (host-side, same DMA bytes as f32):
    hs = hi + lo/2^11, 64*w = whi + wlo/2^11, with each part fp16 (11-bit
    mantissa, so ~22 mantissa bits total; the dropped lo*lo term is ~2^-22).
    The scaling keeps the lo parts in fp16 normal range.  Native fp32 matmul
    would be 4 cycles/row and trips a walrus codegen limit on sync waits for
    self-loading fp32 LDWEIGHTS; fp16 runs 1 cycle/row.
  - Per k-tile only TWO matmuls: rhs = [whi | wlo] concatenated [128 x 512]
    shares one weight load for the hi*hi and hi*lo terms; the lo*hi term
    accumulates into the same scaled-2^11 PSUM columns as hi*lo:
        psum[:, 0:256]   += hshi . whi
        psum[:, 256:512] += hshi . wlo + hslo . whi
    logits = 2^-6 * psum[:, 0:256] + 2^-17 * psum[:, 256:512]
  - DVE max/max_index give the top-8 values + indices per token in one
    instruction each.  Softmax over the full 256 experts followed by top-k
    renormalization reduces algebraically to a softmax over just the top-8
    logits, so the full-row softmax is never materialized.
"""

import numpy as np

TOP_K = 8
NUM_EXPERTS = 256
HIDDEN = 7168
NUM_TOKENS = 16384
N_CORES = 8
T_LOC = NUM_TOKENS // N_CORES

W_SCALE = 64.0       # weight pre-scale so fp16(64*w) stays normal-range
LO_SCALE = 2048.0    # 2^11: lo parts carry the next 11 mantissa bits

_NC_CACHE = {}


def build_gate_nc(t_loc=T_LOC, h=HIDDEN, e=NUM_EXPERTS, repeat=1):
    import concourse.mybir as mybir
    import concourse.tile as tile
    from concourse import bacc

    f32 = mybir.dt.float32
    fp16 = mybir.dt.float16
    P = 128
    KT = h // P          # k-tiles along hidden dim
    TS = t_loc // P      # 128-token subtiles per core
    KC = 8 if KT % 8 == 0 else (4 if KT % 4 == 0 else 1)  # k-tiles per DMA
    NKC = KT // KC       # number of k-chunks

    nc = bacc.Bacc("TRN2", target_bir_lowering=False)
    hsT_hi = nc.dram_tensor("hsT_hi", [h, t_loc], fp16, kind="ExternalInput")
    hsT_lo = nc.dram_tensor("hsT_lo", [h, t_loc], fp16, kind="ExternalInput")
    # wT_cat[:, 0:e] = fp16(64*wT), wT_cat[:, e:2e] = fp16((64*wT - hi) * 2^11)
    wT_cat = nc.dram_tensor("wT_cat", [h, 2 * e], fp16, kind="ExternalInput")
    idx_out = nc.dram_tensor(
        "topk_idx", [t_loc, TOP_K], mybir.dt.int32, kind="ExternalOutput"
    )
    w_out = nc.dram_tensor("topk_w", [t_loc, TOP_K], f32, kind="ExternalOutput")

    # [128, KT, *] views with H split over partitions
    hshi_t = hsT_hi[:, :].rearrange("(ko p) t -> p ko t", p=P)
    hslo_t = hsT_lo[:, :].rearrange("(ko p) t -> p ko t", p=P)
    wcat_t = wT_cat[:, :].rearrange("(ko p) e -> p ko e", p=P)

    with tile.TileContext(nc) as tc:
        with (
            tc.tile_pool(name="wpool", bufs=1) as wpool,
            tc.tile_pool(name="hpool", bufs=28) as hpool,
            tc.tile_pool(name="lpool", bufs=3) as lpool,
            tc.tile_pool(name="spool", bufs=4) as spool,
            tc.tile_pool(name="psum", bufs=4, space="PSUM") as psum_pool,
        ):
            # output staging: small per-tile results accumulate here and leave
            # as two large descriptor DMAs at the end (tiny per-tile DMAs get
            # the DIRECT2D encoding whose single wait slot walrus overflows)
            stage_idx = wpool.tile([P, TS, TOP_K], mybir.dt.int32, tag="sidx")
            stage_wv = wpool.tile([P, TS, TOP_K], f32, tag="swv")
            # gate weight: resident in SBUF, one tile per k-chunk so each
            # matmul depends on exactly one weight-load DMA
            wt_chunks = []
            for kc in range(NKC):
                wc = wpool.tile([P, KC, 2 * e], fp16, tag=f"wt{kc}", name=f"wt{kc}")
                nc.sync.dma_start(wc, wcat_t[:, kc * KC : (kc + 1) * KC, :])
                wt_chunks.append(wc)
            for rep in range(repeat):
                for ts_i in range(TS):
                    tslc = slice(ts_i * P, (ts_i + 1) * P)
                    hs_chunks = []
                    for kc in range(NKC):
                        kslc = slice(kc * KC, (kc + 1) * KC)
                        hhi = hpool.tile(
                            [P, KC, P], fp16, tag="hs", name=f"hshi{rep}_{ts_i}_{kc}"
                        )
                        nc.sync.dma_start(hhi, hshi_t[:, kslc, tslc])
                        hlo = hpool.tile(
                            [P, KC, P], fp16, tag="hs", name=f"hslo{rep}_{ts_i}_{kc}"
                        )
                        nc.sync.dma_start(hlo, hslo_t[:, kslc, tslc])
                        hs_chunks.append((hhi, hlo))
                    pt = psum_pool.tile([P, 2 * e], f32, tag="pt")
                    for k in range(KT):
                        kc, ki = divmod(k, KC)
                        hhi, hlo = hs_chunks[kc]
                        wc = wt_chunks[kc]
                        # psum[:, 0:2e] += hshi . [whi | wlo]
                        nc.tensor.matmul(
                            pt,
                            hhi[:, ki, :],
                            wc[:, ki, :],
                            start=(k == 0),
                            stop=False,
                        )
                        # psum[:, e:2e] += hslo . whi   (same 2^11 scale as hi*lo)
                        nc.tensor.matmul(
                            pt[:, e:],
                            hlo[:, ki, :],
                            wc[:, ki, :e],
                            start=False,
                            stop=(k == KT - 1),
                        )
                    # logits = 2^-6 * psum_hi + 2^-17 * psum_cross
                    cross = lpool.tile([P, e], f32, tag="cross")
                    nc.vector.tensor_scalar_mul(cross, pt[:, e:], 1.0 / (64.0 * 2048.0))
                    logits = lpool.tile([P, e], f32, tag="logits")
                    nc.vector.tensor_scalar(
                        logits,
                        pt[:, :e],
                        1.0 / 64.0,
                        None,
                        mybir.AluOpType.mult,
                    )
                    nc.vector.tensor_add(logits, logits, cross)
                    mx = spool.tile([P, TOP_K], f32, tag="mx")
                    nc.vector.max(out=mx, in_=logits)
                    idx_u = spool.tile([P, TOP_K], mybir.dt.uint32, tag="idxu")
                    nc.vector.max_index(idx_u, mx, logits)
                    nc.vector.tensor_copy(stage_idx[:, ts_i, :], idx_u)
                    # normalized top-k softmax: exp(v - v_max) / sum
                    nm = spool.tile([P, 1], f32, tag="nm")
                    nc.vector.tensor_scalar_mul(nm, mx[:, 0:1], -1.0)
                    ev = spool.tile([P, TOP_K], f32, tag="ev")
                    sm = spool.tile([P, 1], f32, tag="sm")
                    nc.scalar.activation(
                        ev,
                        mx,
                        mybir.ActivationFunctionType.Exp,
                        bias=nm,
                        scale=1.0,
                        accum_out=sm,
                    )
                    rc = spool.tile([P, 1], f32, tag="rc")
                    nc.vector.reciprocal(rc, sm)
                    nc.vector.tensor_scalar_mul(stage_wv[:, ts_i, :], ev, rc)
            nc.sync.dma_start(
                idx_out[:, :].rearrange("(ts p) k -> p ts k", p=P), stage_idx
            )
            nc.sync.dma_start(
                w_out[:, :].rearrange("(ts p) k -> p ts k", p=P), stage_wv
            )
    nc.compile()
    return nc


def _get_nc():
    key = (T_LOC, HIDDEN, NUM_EXPERTS)
    if key not in _NC_CACHE:
        _NC_CACHE[key] = build_gate_nc(*key)
    return _NC_CACHE[key]


def _split_fp16(x, pre_scale=1.0):
    """x (f32) -> (hi, lo) fp16 with hi + lo/2^11 ~= pre_scale*x."""
    xs = x * np.float32(pre_scale) if pre_scale != 1.0 else x
    hi = xs.astype(np.float16)
    lo = ((xs - hi.astype(np.float32)) * np.float32(LO_SCALE)).astype(np.float16)
    return hi, lo


def _prep_inputs(hs, w):
    wT = np.ascontiguousarray(w.T)  # [H, E]
    w_hi, w_lo = _split_fp16(wT, W_SCALE)
    wT_cat = np.concatenate([w_hi, w_lo], axis=1)  # [H, 2E]
    in_maps = []
    for c in range(N_CORES):
        hsT_c = np.ascontiguousarray(hs[c * T_LOC : (c + 1) * T_LOC].T)  # [H, T_LOC]
        hs_hi, hs_lo = _split_fp16(hsT_c)
        in_maps.append({"hsT_hi": hs_hi, "hsT_lo": hs_lo, "wT_cat": wT_cat})
    return in_maps


_FN_CACHE = {}


def _make_runner(nc):
    """Compile a reusable 8-core PJRT callable (same lowering path as
    run_bass_kernel_spmd under axon, but cached so repeat kernel() calls
    skip re-tracing/compiling)."""
    import jax
    import concourse.mybir as mybir
    from concourse import bass2jax
    from jax.sharding import Mesh, NamedSharding, PartitionSpec
    from jax.experimental.shard_map import shard_map

    bass2jax.install_neuronx_cc_hook()
    partition_name = nc.partition_id_tensor.name if nc.partition_id_tensor else None
    in_names, out_names, out_avals, zero_shapes = [], [], [], []
    for alloc in nc.m.functions[0].allocations:
        if not isinstance(alloc, mybir.MemoryLocationSet):
            continue
        name = alloc.memorylocations[0].name
        if alloc.kind == "ExternalInput":
            if name != partition_name:
                in_names.append(name)
        elif alloc.kind == "ExternalOutput":
            shape = tuple(alloc.tensor_shape)
            dtype = mybir.dt.np(alloc.dtype)
            out_names.append(name)
            out_avals.append(jax.core.ShapedArray(shape, dtype))
            zero_shapes.append((shape, dtype))
    n_params = len(in_names)
    n_outs = len(out_avals)
    all_in_names = list(in_names) + list(out_names)
    if partition_name is not None:
        all_in_names.append(partition_name)

    def _body(*args):
        operands = list(args)
        if partition_name is not None:
            operands.append(bass2jax.partition_id_tensor())
        outs = bass2jax._bass_exec_p.bind(
            *operands,
            out_avals=tuple(out_avals),
            in_names=tuple(all_in_names),
            out_names=tuple(out_names),
            lowering_input_output_aliases=(),
            sim_require_finite=True,
            sim_require_nnan=True,
            nc=nc,
        )
        return tuple(outs)

    devices = jax.devices()[:N_CORES]
    mesh = Mesh(np.asarray(devices), ("core",))
    in_specs = (PartitionSpec("core"),) * (n_params + n_outs)
    out_specs = (PartitionSpec("core"),) * len(out_names)
    donate = tuple(range(n_params, n_params + n_outs))
    fn = jax.jit(
        shard_map(
            _body, mesh=mesh, in_specs=in_specs, out_specs=out_specs, check_rep=False
        ),
        donate_argnums=donate,
        keep_unused=True,
    )
    sharding = NamedSharding(mesh, PartitionSpec("core"))

    def run(in_maps):
        concat_in = [
            np.concatenate(
                [np.asarray(in_maps[c][nm]) for c in range(N_CORES)], axis=0
            )
            for nm in in_names
        ]
        zeros = [
            np.zeros((N_CORES * s[0], *s[1:]), dt) for s, dt in zero_shapes
        ]
        dev_in = [jax.device_put(x, sharding) for x in concat_in]
        out_arrs = fn(*dev_in, *zeros)
        return [
            {
                nm: np.asarray(out_arrs[i]).reshape(
                    N_CORES, *out_avals[i].shape
                )[c]
                for i, nm in enumerate(out_names)
            }
            for c in range(N_CORES)
        ]

    return run


def kernel(hidden_states, weight):
    hs = np.asarray(hidden_states, dtype=np.float32)
    w = np.asarray(weight, dtype=np.float32)
    assert hs.shape == (NUM_TOKENS, HIDDEN), hs.shape
    assert w.shape == (NUM_EXPERTS, HIDDEN), w.shape

    in_maps = _prep_inputs(hs, w)
    nc = _get_nc()
    try:
        if "run" not in _FN_CACHE:
            _FN_CACHE["run"] = _make_runner(nc)
        results = _FN_CACHE["run"](in_maps)
    except Exception:
        # fall back to the stock path if the cached-runner path breaks
        from concourse.bass_utils import run_bass_kernel_spmd

        results = run_bass_kernel_spmd(
            nc, in_maps, core_ids=list(range(N_CORES))
        ).results

    topk_idx = np.concatenate([r["topk_idx"] for r in results], axis=0)
    topk_w = np.concatenate([r["topk_w"] for r in results], axis=0)
    row_idx = (
        np.arange(NUM_TOKENS * TOP_K, dtype=np.int32).reshape(TOP_K, NUM_TOKENS).T
    )
    return (
        topk_idx.astype(np.int32),
        topk_w.astype(np.float32),
        row_idx,
    )



# revision 2
# speedup vs baseline: 94.5969x; 94.5969x over previous
"""MoE gate (router) kernel for Trainium2 — v2: big-line DMA layout.

Computes, for hidden_states [T, H] and gate weight [E, H]:
    logits = hidden_states @ weight.T          # [T, E]
    probs  = softmax(logits, axis=-1)
    topk_weight, topk_idx = top_k(probs, 8)    # normalized over the top-8
    row_idx = arange(T*8).reshape(8, T).T

Strategy (8 NeuronCores, data parallel over tokens):
  - fp32 accuracy from fp16 hi/lo splits (host-side, same DMA bytes as f32):
    hs = hi + lo/2^11, 64*w = whi + wlo/2^11.  Three fp16 matmul terms:
        psum[:, 0:512]   += hshi . [whi | wlo]
        psum[:, 256:512] += hslo . whi          (same 2^11 scale as hi*lo)
    logits = 2^-6 * psum[:, 0:256] + 2^-17 * psum[:, 256:512]
  - v2 DMA layout: the baseline DMA'd [128, KC, 128-token] tiles whose
    256-byte contiguous lines pay the <512B half-bandwidth DMA penalty
    (NTFF: DMA 98% active at ~187GB/s, 57us of PE idle).  v2 pre-arranges
    hs host-side as [p=128][half][ko][1024 tokens] so each k-chunk DMA
    moves 16KB-contiguous per-partition lines at full bandwidth, and
    splits hi (SP queue) / lo (Activation queue) across both HWDGE rings.
  - Tokens are processed in two 1024-token halves; each half accumulates
    8 x 128-token subtiles in the 8 PSUM banks across all 56 k-tiles.
  - DVE max/max_index give top-8 values+indices; softmax over the full
    256 experts + top-k renorm reduces to a softmax over the top-8 logits.
"""

import numpy as np

TOP_K = 8
NUM_EXPERTS = 256
HIDDEN = 7168
NUM_TOKENS = 16384
N_CORES = 8
T_LOC = NUM_TOKENS // N_CORES

W_SCALE = 64.0       # weight pre-scale so fp16(64*w) stays normal-range
LO_SCALE = 2048.0    # 2^11: lo parts carry the next 11 mantissa bits

P = 128
KT = HIDDEN // P     # 56 k-tiles along hidden dim
T_HALF = T_LOC // 2  # 1024 tokens per half-pass
TS_HALF = T_HALF // P  # 8 subtiles per half = 8 PSUM banks
KC = 8               # k-tiles per DMA chunk (16KB/partition lines)
NKC = KT // KC       # 7 chunks

_NC_CACHE = {}


def build_gate_nc(t_loc=T_LOC, h=HIDDEN, e=NUM_EXPERTS, repeat=1):
    import concourse.mybir as mybir
    import concourse.tile as tile
    from concourse import bacc

    f32 = mybir.dt.float32
    fp16 = mybir.dt.float16

    nc = bacc.Bacc("TRN2", target_bir_lowering=False)
    # [p][half][ko][t]: per-partition contiguous KC*1024 fp16 = 16KB chunks
    hsT_hi = nc.dram_tensor("hsT_hi", [P, 2 * KT * T_HALF], fp16, kind="ExternalInput")
    hsT_lo = nc.dram_tensor("hsT_lo", [P, 2 * KT * T_HALF], fp16, kind="ExternalInput")
    # [p][ko][0:256]=fp16(64*wT), [p][ko][256:512]=fp16((64*wT - hi) * 2^11)
    wT_cat = nc.dram_tensor("wT_cat", [P, KT * 2 * e], fp16, kind="ExternalInput")
    idx_out = nc.dram_tensor(
        "topk_idx", [t_loc, TOP_K], mybir.dt.int32, kind="ExternalOutput"
    )
    w_out = nc.dram_tensor("topk_w", [t_loc, TOP_K], f32, kind="ExternalOutput")

    TS = t_loc // P  # 16 subtiles total

    with tile.TileContext(nc) as tc:
        with (
            tc.tile_pool(name="wpool", bufs=1) as wpool,
            tc.tile_pool(name="hpool", bufs=4) as hpool,
            tc.tile_pool(name="lpool", bufs=3) as lpool,
            tc.tile_pool(name="spool", bufs=4) as spool,
            tc.tile_pool(name="psum", bufs=8, space="PSUM") as psum_pool,
        ):
            # output staging: results accumulate here and leave as two large
            # descriptor DMAs at the end
            stage_idx = wpool.tile([P, TS, TOP_K], mybir.dt.int32, tag="sidx")
            stage_wv = wpool.tile([P, TS, TOP_K], f32, tag="swv")
            # gate weight: resident in SBUF, one 56KB-per-partition DMA
            wt = wpool.tile([P, KT, 2 * e], fp16, tag="wt")
            nc.gpsimd.dma_start(wt, wT_cat[:, :].rearrange("p (ko e) -> p ko e", ko=KT))
            for rep in range(repeat):
                for half in range(2):
                    pts = []
                    for ts_i in range(TS_HALF):
                        pts.append(
                            psum_pool.tile(
                                [P, 2 * e], f32, tag="pt",
                                name=f"pt{rep}_{half}_{ts_i}",
                            )
                        )
                    for kc in range(NKC):
                        base = half * (KT * T_HALF) + kc * (KC * T_HALF)
                        hhi = hpool.tile(
                            [P, KC, T_HALF], fp16, tag="hs",
                            name=f"hshi{rep}_{half}_{kc}",
                        )
                        nc.sync.dma_start(
                            hhi,
                            hsT_hi[:, base : base + KC * T_HALF].rearrange(
                                "p (ko t) -> p ko t", ko=KC
                            ),
                        )
                        hlo = hpool.tile(
                            [P, KC, T_HALF], fp16, tag="hs",
                            name=f"hslo{rep}_{half}_{kc}",
                        )
                        nc.scalar.dma_start(
                            hlo,
                            hsT_lo[:, base : base + KC * T_HALF].rearrange(
                                "p (ko t) -> p ko t", ko=KC
                            ),
                        )
                        for ki in range(KC):
                            k = kc * KC + ki
                            for ts_i in range(TS_HALF):
                                pt = pts[ts_i]
                                tslc = slice(ts_i * P, (ts_i + 1) * P)
                                # psum[:, 0:2e] += hshi . [whi | wlo]
                                nc.tensor.matmul(
                                    pt,
                                    hhi[:, ki, tslc],
                                    wt[:, k, :],
                                    start=(k == 0),
                                    stop=False,
                                )
                                # psum[:, e:2e] += hslo . whi  (2^11 scale)
                                nc.tensor.matmul(
                                    pt[:, e:],
                                    hlo[:, ki, tslc],
                                    wt[:, k, :e],
                                    start=False,
                                    stop=(k == KT - 1),
                                )
                    for ts_i in range(TS_HALF):
                        pt = pts[ts_i]
                        g_ts = half * TS_HALF + ts_i
                        # logits = 2^-6 * psum_hi + 2^-17 * psum_cross
                        cross = lpool.tile([P, e], f32, tag="cross")
                        nc.vector.tensor_scalar_mul(
                            cross, pt[:, e:], 1.0 / (64.0 * 2048.0)
                        )
                        logits = lpool.tile([P, e], f32, tag="logits")
                        nc.vector.tensor_scalar(
                            logits,
                            pt[:, :e],
                            1.0 / 64.0,
                            None,
                            mybir.AluOpType.mult,
                        )
                        nc.vector.tensor_add(logits, logits, cross)
                        mx = spool.tile([P, TOP_K], f32, tag="mx")
                        nc.vector.max(out=mx, in_=logits)
                        idx_u = spool.tile([P, TOP_K], mybir.dt.uint32, tag="idxu")
                        nc.vector.max_index(idx_u, mx, logits)
                        nc.vector.tensor_copy(stage_idx[:, g_ts, :], idx_u)
                        # normalized top-k softmax: exp(v - v_max) / sum
                        nm = spool.tile([P, 1], f32, tag="nm")
                        nc.vector.tensor_scalar_mul(nm, mx[:, 0:1], -1.0)
                        ev = spool.tile([P, TOP_K], f32, tag="ev")
                        sm = spool.tile([P, 1], f32, tag="sm")
                        nc.scalar.activation(
                            ev,
                            mx,
                            mybir.ActivationFunctionType.Exp,
                            bias=nm,
                            scale=1.0,
                            accum_out=sm,
                        )
                        rc = spool.tile([P, 1], f32, tag="rc")
                        nc.vector.reciprocal(rc, sm)
                        nc.vector.tensor_scalar_mul(stage_wv[:, g_ts, :], ev, rc)
            nc.sync.dma_start(
                idx_out[:, :].rearrange("(ts p) k -> p ts k", p=P), stage_idx
            )
            nc.sync.dma_start(
                w_out[:, :].rearrange("(ts p) k -> p ts k", p=P), stage_wv
            )
    nc.compile()
    return nc


def _get_nc():
    key = (T_LOC, HIDDEN, NUM_EXPERTS)
    if key not in _NC_CACHE:
        _NC_CACHE[key] = build_gate_nc(*key)
    return _NC_CACHE[key]


def _split_fp16(x, pre_scale=1.0):
    """x (f32) -> (hi, lo) fp16 with hi + lo/2^11 ~= pre_scale*x."""
    xs = x * np.float32(pre_scale) if pre_scale != 1.0 else x
    hi = xs.astype(np.float16)
    lo = ((xs - hi.astype(np.float32)) * np.float32(LO_SCALE)).astype(np.float16)
    return hi, lo


def _pack_hs(hs_part):
    """[t_loc, H] f32 -> [128, 2*56*1024] fp16 hi/lo in [p][half][ko][t] order."""
    hsT = np.ascontiguousarray(hs_part.T)  # [H, t_loc]
    hi, lo = _split_fp16(hsT)
    out = []
    for x in (hi, lo):
        x4 = x.reshape(KT, P, 2, T_HALF)          # [ko][p][half][t]
        x4 = np.ascontiguousarray(x4.transpose(1, 2, 0, 3))  # [p][half][ko][t]
        out.append(x4.reshape(P, 2 * KT * T_HALF))
    return out


def _prep_inputs(hs, w):
    wT = np.ascontiguousarray(w.T)  # [H, E]
    w_hi, w_lo = _split_fp16(wT, W_SCALE)
    w_cat = np.concatenate([w_hi, w_lo], axis=1)          # [H, 2E]
    w3 = w_cat.reshape(KT, P, 2 * NUM_EXPERTS)            # [ko][p][2e]
    w3 = np.ascontiguousarray(w3.transpose(1, 0, 2))      # [p][ko][2e]
    wT_packed = w3.reshape(P, KT * 2 * NUM_EXPERTS)
    in_maps = []
    for c in range(N_CORES):
        hs_hi, hs_lo = _pack_hs(hs[c * T_LOC : (c + 1) * T_LOC])
        in_maps.append({"hsT_hi": hs_hi, "hsT_lo": hs_lo, "wT_cat": wT_packed})
    return in_maps


_FN_CACHE = {}


def _make_runner(nc):
    """Compile a reusable 8-core PJRT callable (same lowering path as
    run_bass_kernel_spmd under axon, but cached so repeat kernel() calls
    skip re-tracing/compiling)."""
    import jax
    import concourse.mybir as mybir
    from concourse import bass2jax
    from jax.sharding import Mesh, NamedSharding, PartitionSpec
    from jax.experimental.shard_map import shard_map

    bass2jax.install_neuronx_cc_hook()
    partition_name = nc.partition_id_tensor.name if nc.partition_id_tensor else None
    in_names, out_names, out_avals, zero_shapes = [], [], [], []
    for alloc in nc.m.functions[0].allocations:
        if not isinstance(alloc, mybir.MemoryLocationSet):
            continue
        name = alloc.memorylocations[0].name
        if alloc.kind == "ExternalInput":
            if name != partition_name:
                in_names.append(name)
        elif alloc.kind == "ExternalOutput":
            shape = tuple(alloc.tensor_shape)
            dtype = mybir.dt.np(alloc.dtype)
            out_names.append(name)
            out_avals.append(jax.core.ShapedArray(shape, dtype))
            zero_shapes.append((shape, dtype))
    n_params = len(in_names)
    n_outs = len(out_avals)
    all_in_names = list(in_names) + list(out_names)
    if partition_name is not None:
        all_in_names.append(partition_name)

    def _body(*args):
        operands = list(args)
        if partition_name is not None:
            operands.append(bass2jax.partition_id_tensor())
        outs = bass2jax._bass_exec_p.bind(
            *operands,
            out_avals=tuple(out_avals),
            in_names=tuple(all_in_names),
            out_names=tuple(out_names),
            lowering_input_output_aliases=(),
            sim_require_finite=True,
            sim_require_nnan=True,
            nc=nc,
        )
        return tuple(outs)

    devices = jax.devices()[:N_CORES]
    mesh = Mesh(np.asarray(devices), ("core",))
    in_specs = (PartitionSpec("core"),) * (n_params + n_outs)
    out_specs = (PartitionSpec("core"),) * len(out_names)
    donate = tuple(range(n_params, n_params + n_outs))
    fn = jax.jit(
        shard_map(
            _body, mesh=mesh, in_specs=in_specs, out_specs=out_specs, check_rep=False
        ),
        donate_argnums=donate,
        keep_unused=True,
    )
    sharding = NamedSharding(mesh, PartitionSpec("core"))

    def run(in_maps):
        concat_in = [
            np.concatenate(
                [np.asarray(in_maps[c][nm]) for c in range(N_CORES)], axis=0
            )
            for nm in in_names
        ]
        zeros = [
            np.zeros((N_CORES * s[0], *s[1:]), dt) for s, dt in zero_shapes
        ]
        dev_in = [jax.device_put(x, sharding) for x in concat_in]
        out_arrs = fn(*dev_in, *zeros)
        return [
            {
                nm: np.asarray(out_arrs[i]).reshape(
                    N_CORES, *out_avals[i].shape
                )[c]
                for i, nm in enumerate(out_names)
            }
            for c in range(N_CORES)
        ]

    return run


def kernel(hidden_states, weight):
    hs = np.asarray(hidden_states, dtype=np.float32)
    w = np.asarray(weight, dtype=np.float32)
    assert hs.shape == (NUM_TOKENS, HIDDEN), hs.shape
    assert w.shape == (NUM_EXPERTS, HIDDEN), w.shape

    in_maps = _prep_inputs(hs, w)
    nc = _get_nc()
    try:
        if "run" not in _FN_CACHE:
            _FN_CACHE["run"] = _make_runner(nc)
        results = _FN_CACHE["run"](in_maps)
    except Exception:
        # fall back to the stock path if the cached-runner path breaks
        from concourse.bass_utils import run_bass_kernel_spmd

        results = run_bass_kernel_spmd(
            nc, in_maps, core_ids=list(range(N_CORES))
        ).results

    topk_idx = np.concatenate([r["topk_idx"] for r in results], axis=0)
    topk_w = np.concatenate([r["topk_w"] for r in results], axis=0)
    row_idx = (
        np.arange(NUM_TOKENS * TOP_K, dtype=np.int32).reshape(TOP_K, NUM_TOKENS).T
    )
    return (
        topk_idx.astype(np.int32),
        topk_w.astype(np.float32),
        row_idx,
    )


# revision 7
# speedup vs baseline: 102.3958x; 1.0824x over previous
"""MoE gate (router) kernel for Trainium2 — v2: big-line DMA layout.

Computes, for hidden_states [T, H] and gate weight [E, H]:
    logits = hidden_states @ weight.T          # [T, E]
    probs  = softmax(logits, axis=-1)
    topk_weight, topk_idx = top_k(probs, 8)    # normalized over the top-8
    row_idx = arange(T*8).reshape(8, T).T

Strategy (8 NeuronCores, data parallel over tokens):
  - fp32 accuracy from fp16 hi/lo splits (host-side, same DMA bytes as f32):
    hs = hi + lo/2^11, 64*w = whi + wlo/2^11.  Three fp16 matmul terms:
        psum[:, 0:512]   += hshi . [whi | wlo]
        psum[:, 256:512] += hslo . whi          (same 2^11 scale as hi*lo)
    logits = 2^-6 * psum[:, 0:256] + 2^-17 * psum[:, 256:512]
  - v2/v3 DMA layout: the baseline DMA'd [128, KC, 128-token] tiles whose
    256-byte contiguous lines pay the <512B half-bandwidth DMA penalty
    (NTFF: DMA 98% active at ~187GB/s, 57us of PE idle).  Now hs is
    pre-arranged host-side as [p=128][group][ko][512 tokens] so each
    k-chunk DMA moves 8KB-contiguous per-partition lines at full
    bandwidth (NTFF: ~304GB/s), with hi on the SP queue and lo on the
    Activation queue across both HWDGE rings.  The gate weight loads in
    NKC chunks on the gpsimd (SWDGE) queue so the first matmul waits on
    ~1MB instead of the whole 7.3MB.
  - Tokens are processed in four 512-token groups; each group accumulates
    4 x 128-token subtiles in 4 PSUM banks across all 56 k-tiles, so two
    groups are in flight and the top-k evacuation of group g overlaps the
    matmuls of group g+1 instead of stalling the PE at group boundaries.
  - DVE max/max_index give top-8 values+indices; softmax over the full
    256 experts + top-k renorm reduces to a softmax over the top-8 logits.
"""

import numpy as np

TOP_K = 8
NUM_EXPERTS = 256
HIDDEN = 7168
NUM_TOKENS = 16384
N_CORES = 8
T_LOC = NUM_TOKENS // N_CORES

W_SCALE = 64.0       # weight pre-scale so fp16(64*w) stays normal-range
LO_SCALE = 2048.0    # 2^11: lo parts carry the next 11 mantissa bits

P = 128
KT = HIDDEN // P     # 56 k-tiles along hidden dim
NG = 4               # token groups per core
T_GRP = T_LOC // NG  # 512 tokens per group
TS_GRP = T_GRP // P  # 4 subtiles per group = 4 PSUM banks (2 groups in flight)
KC = 8               # k-tiles per DMA chunk (8KB/partition lines)
NKC = KT // KC       # 7 chunks

_NC_CACHE = {}


def build_gate_nc(t_loc=T_LOC, h=HIDDEN, e=NUM_EXPERTS, repeat=1):
    import concourse.mybir as mybir
    import concourse.tile as tile
    from concourse import bacc

    f32 = mybir.dt.float32
    fp16 = mybir.dt.float16

    nc = bacc.Bacc("TRN2", target_bir_lowering=False)
    # [p][group][ko][t]: per-partition contiguous KC*512 fp16 = 8KB chunks
    hsT_hi = nc.dram_tensor("hsT_hi", [P, NG * KT * T_GRP], fp16, kind="ExternalInput")
    hsT_lo = nc.dram_tensor("hsT_lo", [P, NG * KT * T_GRP], fp16, kind="ExternalInput")
    # [p][ko][0:256]=fp16(64*wT), [p][ko][256:512]=fp16((64*wT - hi) * 2^11)
    wT_cat = nc.dram_tensor("wT_cat", [P, KT * 2 * e], fp16, kind="ExternalInput")
    idx_out = nc.dram_tensor(
        "topk_idx", [t_loc, TOP_K], mybir.dt.int32, kind="ExternalOutput"
    )
    w_out = nc.dram_tensor("topk_w", [t_loc, TOP_K], f32, kind="ExternalOutput")

    TS = t_loc // P  # 16 subtiles total

    with tile.TileContext(nc) as tc:
        with (
            tc.tile_pool(name="wpool", bufs=1) as wpool,
            tc.tile_pool(name="hpool", bufs=4) as hpool,
            tc.tile_pool(name="lpool", bufs=3) as lpool,
            tc.tile_pool(name="spool", bufs=4) as spool,
            tc.tile_pool(name="psum", bufs=8, space="PSUM") as psum_pool,
        ):
            # output staging: results accumulate here and leave as two large
            # descriptor DMAs at the end
            stage_idx = wpool.tile([P, TS, TOP_K], mybir.dt.int32, tag="sidx")
            stage_wv = wpool.tile([P, TS, TOP_K], f32, tag="swv")
            # gate weight: resident in SBUF, loaded in NKC chunks on the
            # gpsimd queue so the first matmul waits on ~1MB, not 7.3MB
            wt = wpool.tile([P, KT, 2 * e], fp16, tag="wt")
            wt_view = wT_cat[:, :].rearrange("p (ko e) -> p ko e", ko=KT)
            for kc in range(NKC):
                kslc = slice(kc * KC, (kc + 1) * KC)
                nc.gpsimd.dma_start(wt[:, kslc, :], wt_view[:, kslc, :])
            for rep in range(repeat):
                for grp in range(NG):
                    pts = []
                    for ts_i in range(TS_GRP):
                        pts.append(
                            psum_pool.tile(
                                [P, 2 * e], f32, tag="pt",
                                name=f"pt{rep}_{grp}_{ts_i}",
                            )
                        )
                    for kc in range(NKC):
                        base = grp * (KT * T_GRP) + kc * (KC * T_GRP)
                        hhi = hpool.tile(
                            [P, KC, T_GRP], fp16, tag="hs",
                            name=f"hshi{rep}_{grp}_{kc}",
                        )
                        nc.sync.dma_start(
                            hhi,
                            hsT_hi[:, base : base + KC * T_GRP].rearrange(
                                "p (ko t) -> p ko t", ko=KC
                            ),
                        )
                        hlo = hpool.tile(
                            [P, KC, T_GRP], fp16, tag="hs",
                            name=f"hslo{rep}_{grp}_{kc}",
                        )
                        nc.scalar.dma_start(
                            hlo,
                            hsT_lo[:, base : base + KC * T_GRP].rearrange(
                                "p (ko t) -> p ko t", ko=KC
                            ),
                        )
                        for ki in range(KC):
                            k = kc * KC + ki
                            for ts_i in range(TS_GRP):
                                pt = pts[ts_i]
                                tslc = slice(ts_i * P, (ts_i + 1) * P)
                                # psum[:, 0:2e] += hshi . [whi | wlo]
                                nc.tensor.matmul(
                                    pt,
                                    hhi[:, ki, tslc],
                                    wt[:, k, :],
                                    start=(k == 0),
                                    stop=False,
                                )
                                # psum[:, e:2e] += hslo . whi  (2^11 scale)
                                nc.tensor.matmul(
                                    pt[:, e:],
                                    hlo[:, ki, tslc],
                                    wt[:, k, :e],
                                    start=False,
                                    stop=(k == KT - 1),
                                )
                    for ts_i in range(TS_GRP):
                        pt = pts[ts_i]
                        g_ts = grp * TS_GRP + ts_i
                        # logits = 2^-6 * psum_hi + 2^-17 * psum_cross
                        cross = lpool.tile([P, e], f32, tag="cross")
                        nc.vector.tensor_scalar_mul(
                            cross, pt[:, e:], 1.0 / (64.0 * 2048.0)
                        )
                        logits = lpool.tile([P, e], f32, tag="logits")
                        nc.vector.tensor_scalar(
                            logits,
                            pt[:, :e],
                            1.0 / 64.0,
                            None,
                            mybir.AluOpType.mult,
                        )
                        nc.vector.tensor_add(logits, logits, cross)
                        mx = spool.tile([P, TOP_K], f32, tag="mx")
                        nc.vector.max(out=mx, in_=logits)
                        idx_u = spool.tile([P, TOP_K], mybir.dt.uint32, tag="idxu")
                        nc.vector.max_index(idx_u, mx, logits)
                        nc.vector.tensor_copy(stage_idx[:, g_ts, :], idx_u)
                        # normalized top-k softmax: exp(v - v_max) / sum
                        nm = spool.tile([P, 1], f32, tag="nm")
                        nc.vector.tensor_scalar_mul(nm, mx[:, 0:1], -1.0)
                        ev = spool.tile([P, TOP_K], f32, tag="ev")
                        sm = spool.tile([P, 1], f32, tag="sm")
                        nc.scalar.activation(
                            ev,
                            mx,
                            mybir.ActivationFunctionType.Exp,
                            bias=nm,
                            scale=1.0,
                            accum_out=sm,
                        )
                        rc = spool.tile([P, 1], f32, tag="rc")
                        nc.vector.reciprocal(rc, sm)
                        nc.vector.tensor_scalar_mul(stage_wv[:, g_ts, :], ev, rc)
            nc.sync.dma_start(
                idx_out[:, :].rearrange("(ts p) k -> p ts k", p=P), stage_idx
            )
            nc.sync.dma_start(
                w_out[:, :].rearrange("(ts p) k -> p ts k", p=P), stage_wv
            )
    nc.compile()
    return nc


def _get_nc():
    key = (T_LOC, HIDDEN, NUM_EXPERTS)
    if key not in _NC_CACHE:
        _NC_CACHE[key] = build_gate_nc(*key)
    return _NC_CACHE[key]


def _split_fp16(x, pre_scale=1.0):
    """x (f32) -> (hi, lo) fp16 with hi + lo/2^11 ~= pre_scale*x."""
    xs = x * np.float32(pre_scale) if pre_scale != 1.0 else x
    hi = xs.astype(np.float16)
    lo = ((xs - hi.astype(np.float32)) * np.float32(LO_SCALE)).astype(np.float16)
    return hi, lo


def _pack_hs(hs_part):
    """[t_loc, H] f32 -> [128, 4*56*512] fp16 hi/lo in [p][group][ko][t] order."""
    hsT = np.ascontiguousarray(hs_part.T)  # [H, t_loc]
    hi, lo = _split_fp16(hsT)
    out = []
    for x in (hi, lo):
        x4 = x.reshape(KT, P, NG, T_GRP)          # [ko][p][group][t]
        x4 = np.ascontiguousarray(x4.transpose(1, 2, 0, 3))  # [p][group][ko][t]
        out.append(x4.reshape(P, NG * KT * T_GRP))
    return out


def _prep_inputs(hs, w):
    wT = np.ascontiguousarray(w.T)  # [H, E]
    w_hi, w_lo = _split_fp16(wT, W_SCALE)
    w_cat = np.concatenate([w_hi, w_lo], axis=1)          # [H, 2E]
    w3 = w_cat.reshape(KT, P, 2 * NUM_EXPERTS)            # [ko][p][2e]
    w3 = np.ascontiguousarray(w3.transpose(1, 0, 2))      # [p][ko][2e]
    wT_packed = w3.reshape(P, KT * 2 * NUM_EXPERTS)
    in_maps = []
    for c in range(N_CORES):
        hs_hi, hs_lo = _pack_hs(hs[c * T_LOC : (c + 1) * T_LOC])
        in_maps.append({"hsT_hi": hs_hi, "hsT_lo": hs_lo, "wT_cat": wT_packed})
    return in_maps


_FN_CACHE = {}


def _make_runner(nc):
    """Compile a reusable 8-core PJRT callable (same lowering path as
    run_bass_kernel_spmd under axon, but cached so repeat kernel() calls
    skip re-tracing/compiling)."""
    import jax
    import concourse.mybir as mybir
    from concourse import bass2jax
    from jax.sharding import Mesh, NamedSharding, PartitionSpec
    from jax.experimental.shard_map import shard_map

    bass2jax.install_neuronx_cc_hook()
    partition_name = nc.partition_id_tensor.name if nc.partition_id_tensor else None
    in_names, out_names, out_avals, zero_shapes = [], [], [], []
    for alloc in nc.m.functions[0].allocations:
        if not isinstance(alloc, mybir.MemoryLocationSet):
            continue
        name = alloc.memorylocations[0].name
        if alloc.kind == "ExternalInput":
            if name != partition_name:
                in_names.append(name)
        elif alloc.kind == "ExternalOutput":
            shape = tuple(alloc.tensor_shape)
            dtype = mybir.dt.np(alloc.dtype)
            out_names.append(name)
            out_avals.append(jax.core.ShapedArray(shape, dtype))
            zero_shapes.append((shape, dtype))
    n_params = len(in_names)
    n_outs = len(out_avals)
    all_in_names = list(in_names) + list(out_names)
    if partition_name is not None:
        all_in_names.append(partition_name)

    def _body(*args):
        operands = list(args)
        if partition_name is not None:
            operands.append(bass2jax.partition_id_tensor())
        outs = bass2jax._bass_exec_p.bind(
            *operands,
            out_avals=tuple(out_avals),
            in_names=tuple(all_in_names),
            out_names=tuple(out_names),
            lowering_input_output_aliases=(),
            sim_require_finite=True,
            sim_require_nnan=True,
            nc=nc,
        )
        return tuple(outs)

    devices = jax.devices()[:N_CORES]
    mesh = Mesh(np.asarray(devices), ("core",))
    in_specs = (PartitionSpec("core"),) * (n_params + n_outs)
    out_specs = (PartitionSpec("core"),) * len(out_names)
    donate = tuple(range(n_params, n_params + n_outs))
    fn = jax.jit(
        shard_map(
            _body, mesh=mesh, in_specs=in_specs, out_specs=out_specs, check_rep=False
        ),
        donate_argnums=donate,
        keep_unused=True,
    )
    sharding = NamedSharding(mesh, PartitionSpec("core"))

    def run(in_maps):
        concat_in = [
            np.concatenate(
                [np.asarray(in_maps[c][nm]) for c in range(N_CORES)], axis=0
            )
            for nm in in_names
        ]
        zeros = [
            np.zeros((N_CORES * s[0], *s[1:]), dt) for s, dt in zero_shapes
        ]
        dev_in = [jax.device_put(x, sharding) for x in concat_in]
        out_arrs = fn(*dev_in, *zeros)
        return [
            {
                nm: np.asarray(out_arrs[i]).reshape(
                    N_CORES, *out_avals[i].shape
                )[c]
                for i, nm in enumerate(out_names)
            }
            for c in range(N_CORES)
        ]

    return run


def kernel(hidden_states, weight):
    hs = np.asarray(hidden_states, dtype=np.float32)
    w = np.asarray(weight, dtype=np.float32)
    assert hs.shape == (NUM_TOKENS, HIDDEN), hs.shape
    assert w.shape == (NUM_EXPERTS, HIDDEN), w.shape

    in_maps = _prep_inputs(hs, w)
    nc = _get_nc()
    try:
        if "run" not in _FN_CACHE:
            _FN_CACHE["run"] = _make_runner(nc)
        results = _FN_CACHE["run"](in_maps)
    except Exception:
        # fall back to the stock path if the cached-runner path breaks
        from concourse.bass_utils import run_bass_kernel_spmd

        results = run_bass_kernel_spmd(
            nc, in_maps, core_ids=list(range(N_CORES))
        ).results

    topk_idx = np.concatenate([r["topk_idx"] for r in results], axis=0)
    topk_w = np.concatenate([r["topk_w"] for r in results], axis=0)
    row_idx = (
        np.arange(NUM_TOKENS * TOP_K, dtype=np.int32).reshape(TOP_K, NUM_TOKENS).T
    )
    return (
        topk_idx.astype(np.int32),
        topk_w.astype(np.float32),
        row_idx,
    )
